# revision 1
# baseline (speedup 1.0000x reference)
"""3-block decoder (causal MHA + full MHA + 4-layer FFN, post-LN) on 8 TRN2 cores.

Sharding (SPMD-uniform across all 8 cores; core = 2*batch + hh):
  - Layer 1 (causal MHA): Megatron tensor-parallel over heads within core
    pairs {2b, 2b+1} (8 heads each), AllReduce of the partial out-projection.
  - Layer 2 (full MHA): same TP-over-heads + AllReduce.
  - FFN + all LayerNorms: duplicated on both pair cores (rowwise work, no comm).
  Host assembles output from the even core of each pair.

Layout: activations feature-major [feat(P), tok] ("FM") so every linear uses
natural-layout weights; V token-major via mapping (a); scores computed
transposed S^T=[k,q]; softmax denominators via a ones-column appended to V
(M=65 AV matmuls); normalization deferred to the O tiles.

All matmuls run in float32r (full-rate fp32-relaxed on the PE; rel err ~1e-4).
"""
import sys, os

for p in ("/opt/trn_rl_repo", "/root/.axon_site/_ro/trn_rl_repo"):
    if os.path.isdir(p) and p not in sys.path:
        sys.path.insert(0, p)

import numpy as np
import concourse.bass as bass
import concourse.bacc as bacc
import concourse.tile as tile
import concourse.mybir as mybir
from concourse import bass_utils
from concourse.masks import make_identity

F32 = mybir.dt.float32
F32R = mybir.dt.float32r
I32 = mybir.dt.int32
Exp = mybir.ActivationFunctionType.Exp
Log = mybir.ActivationFunctionType.Ln if hasattr(mybir.ActivationFunctionType, "Ln") else mybir.ActivationFunctionType.Log
Square = mybir.ActivationFunctionType.Square
ADD = mybir.AluOpType.add
SUB = mybir.AluOpType.subtract
MULT = mybir.AluOpType.mult
MAX = mybir.AluOpType.max

B, E, H, HD, V = 4, 1024, 16, 64, 32000
HL = 8          # local heads per core (H/2)
P = 128
EPS = 1e-5


def _f32r(ap):
    return ap.bitcast(F32R)


def _f32(ap):
    return ap.bitcast(F32)


def build_program(T, n_cores, fake_cc=False):
    """One SPMD program for all cores. T = sequence length."""
    NC = E // P          # 8 feature chunks
    NCH = 512 // P       # 4 chunks of the local-head dim (8 heads * 64)
    NB = T // 512        # 512-token blocks
    NTT = T // P         # 128-token tiles
    groups = [[2 * i, 2 * i + 1] for i in range(n_cores // 2)]

    nc = bacc.Bacc("TRN2", target_bir_lowering=False, debug=False,
                   enable_asserts=False, num_devices=n_cores)

    dt_ = nc.dram_tensor
    ids = dt_("ids", [T, 1], I32, kind="ExternalInput").ap()
    tok_emb = dt_("tok_emb", [V, E], F32, kind="ExternalInput").ap()
    pos_emb = dt_("pos_emb", [T, E], F32, kind="ExternalInput").ap()
    wq1 = dt_("wq1", [E, 512], F32R, kind="ExternalInput").ap()
    wk1 = dt_("wk1", [E, 512], F32R, kind="ExternalInput").ap()
    wv1 = dt_("wv1", [E, 512], F32R, kind="ExternalInput").ap()
    wo1 = dt_("wo1", [512, E], F32R, kind="ExternalInput").ap()
    wq2 = dt_("wq2", [E, 512], F32R, kind="ExternalInput").ap()
    wk2 = dt_("wk2", [E, 512], F32R, kind="ExternalInput").ap()
    wv2 = dt_("wv2", [E, 512], F32R, kind="ExternalInput").ap()
    wo2 = dt_("wo2", [512, E], F32R, kind="ExternalInput").ap()
    fw1 = dt_("fw1", [E, 512], F32R, kind="ExternalInput").ap()
    fw2 = dt_("fw2", [512, E], F32R, kind="ExternalInput").ap()
    fw3 = dt_("fw3", [E, 512], F32R, kind="ExternalInput").ap()
    fw4 = dt_("fw4", [512, E], F32R, kind="ExternalInput").ap()
    bo1h = dt_("bo1h", [E], F32, kind="ExternalInput").ap()    # m_bo/2
    bo2h = dt_("bo2h", [E], F32, kind="ExternalInput").ap()    # (h_bo+ln_b)/2
    kb2 = dt_("kb2", [512], F32, kind="ExternalInput").ap()    # ln_b @ h_Wk slice
    qb2 = dt_("qb2", [512], F32, kind="ExternalInput").ap()
    vb2 = dt_("vb2", [512], F32, kind="ExternalInput").ap()
    fb1c = dt_("fb1c", [512], F32, kind="ExternalInput").ap()  # ln_b@f_W1+f_b1
    fb2c = dt_("fb2c", [E], F32, kind="ExternalInput").ap()
    fb3c = dt_("fb3c", [512], F32, kind="ExternalInput").ap()
    fb4c = dt_("fb4c", [E], F32, kind="ExternalInput").ap()    # f_b4 + ln_b
    gvec = dt_("gvec", [E], F32, kind="ExternalInput").ap()    # ln_g
    lnbv = dt_("lnbv", [E], F32, kind="ExternalInput").ap()    # ln_b
    masks = dt_("masks", [4, P, 512], F32, kind="ExternalInput").ap()
    ones_col = dt_("ones_col", [P, 1], F32R, kind="ExternalInput").ap()
    ones_row = dt_("ones_row", [1, P], F32, kind="ExternalInput").ap()
    out = dt_("out", [T, E], F32, kind="ExternalOutput").ap()

    def cp(w):  # [K, N] -> [p, K/128, N] for chunked lhsT loads
        return w.rearrange("(c p) n -> p c n", p=P)

    def vchunk(v):  # [F] -> [p, F/128] (chunk-major per-partition consts)
        return v.rearrange("(c p) -> p c", p=P)

    with tile.TileContext(nc) as tc:
      with nc.allow_low_precision(reason="f32r compute by design"), \
           tc.tile_pool(name="const", bufs=1) as const, \
           tc.tile_pool(name="dram", bufs=1, space="DRAM") as dram:

        # ---- constants resident in SBUF ----
        ident = const.tile([P, P], F32)
        make_identity(nc, ident[:])
        ones_c = const.tile([P, 1], F32R)
        nc.sync.dma_start(ones_c[:], ones_col)
        ones_r = const.tile([1, P], F32)
        nc.sync.dma_start(ones_r[:], ones_row)
        ones_rr = const.tile([1, P], F32R)
        nc.vector.tensor_copy(ones_rr[:], ones_c[0:1, 0:1].to_broadcast([1, P]))
        g_sb = const.tile([P, NC], F32)
        nc.sync.dma_start(g_sb[:], vchunk(gvec))
        lnb_sb = const.tile([P, NC], F32)
        nc.sync.dma_start(lnb_sb[:], vchunk(lnbv))
        bo1_sb = const.tile([P, NC], F32)
        nc.sync.dma_start(bo1_sb[:], vchunk(bo1h))
        bo2_sb = const.tile([P, NC], F32)
        nc.sync.dma_start(bo2_sb[:], vchunk(bo2h))
        kb2_sb = const.tile([P, NCH], F32)
        nc.sync.dma_start(kb2_sb[:], vchunk(kb2))
        qb2_sb = const.tile([P, NCH], F32)
        nc.sync.dma_start(qb2_sb[:], vchunk(qb2))
        fb1_sb = const.tile([P, NCH], F32)
        nc.sync.dma_start(fb1_sb[:], vchunk(fb1c))
        fb2_sb = const.tile([P, NC], F32)
        nc.sync.dma_start(fb2_sb[:], vchunk(fb2c))
        fb3_sb = const.tile([P, NCH], F32)
        nc.sync.dma_start(fb3_sb[:], vchunk(fb3c))
        fb4_sb = const.tile([P, NC], F32)
        nc.sync.dma_start(fb4_sb[:], vchunk(fb4c))
        eps_t = const.tile([1, 1], F32)
        nc.vector.memset(eps_t[:], EPS)
        # vb2 broadcast [p, 8, 64] (per-feature bias of token-major V2)
        vb2_b = const.tile([P, HL * HD], F32)
        nc.sync.dma_start(vb2_b[:], bass.AP(tensor=vb2.tensor, offset=vb2.offset,
                                            ap=[[0, P], [1, HL * HD]]))

        # ---- internal DRAM ----
        e_dram = [dram.tile([P, NC, 512], F32R, name=f"ed{q}")
                  for q in range(NB)]
        n1_dram = [dram.tile([P, NC, 512], F32R, name=f"n1d{q}")
                   for q in range(NB)]
        q_dram = [[dram.tile([P, NCH, 512], F32R, name=f"qd{i}_{q}")
                   for q in range(NB)] for i in range(2)]
        attn_p = [[dram.tile([P, NC, 512], F32, name=f"ap{i}_{q}")
                   for q in range(NB)] for i in range(2)]
        attn_f = [[dram.tile([P, NC, 512], F32, name=f"af{i}_{q}")
                   for q in range(NB)] for i in range(2)]

        # ================= embeddings: gather + pos + transpose to FM ========
        with tc.tile_pool(name="emb", bufs=3) as emb_p, \
             tc.tile_pool(name="emb_ps", bufs=4, space="PSUM") as emb_ps:
            for tt in range(NTT):
                ids_t = emb_p.tile([P, 1], I32, tag="ids")
                nc.sync.dma_start(ids_t[:], ids[tt * P:(tt + 1) * P, :])
                et = emb_p.tile([P, E], F32, tag="emb")
                nc.gpsimd.indirect_dma_start(
                    out=et[:], out_offset=None, in_=tok_emb[:],
                    in_offset=bass.IndirectOffsetOnAxis(ap=ids_t[:, :1], axis=0))
                pt = emb_p.tile([P, E], F32, tag="pos")
                nc.sync.dma_start(pt[:], pos_emb[tt * P:(tt + 1) * P, :])
                nc.vector.tensor_tensor(et[:], et[:], pt[:], ADD)
                for c in range(NC):
                    tp = emb_ps.tile([P, P], F32, tag="tp")
                    nc.tensor.transpose(tp[:], et[:, c * P:(c + 1) * P], ident[:])
                    est = emb_p.tile([P, P], F32R, tag="est", bufs=4)
                    nc.vector.tensor_copy(est[:], tp[:])
                    nc.sync.dma_start(
                        e_dram[tt // 4][:, c, (tt % 4) * P:(tt % 4 + 1) * P], est[:])

        # ======== shared phase builders =====================================
        def proj_pass(src_dram, wk, wq, wv, K_sb, V_sb, qd, kbias, qbias, vbias,
                      pre_w=None):
            """Stream src (FM, DRAM) once; produce K (SBUF FM), Q (DRAM FM),
            V (SBUF TM + ones col). pre_w = preloaded (wk_sb, wq_sb, wv_sb)."""
            with tc.tile_pool(name="pw", bufs=1) as pw, \
                 tc.tile_pool(name="px", bufs=2) as px, \
                 tc.tile_pool(name="pst", bufs=3) as pst, \
                 tc.tile_pool(name="pps", bufs=4, space="PSUM") as pps:
                if pre_w is not None:
                    wk_sb, wq_sb, wv_sb = pre_w
                else:
                    wk_sb = pw.tile([P, NC, 512], F32R)
                    nc.sync.dma_start(wk_sb[:], cp(wk))
                    wq_sb = pw.tile([P, NC, 512], F32R)
                    nc.sync.dma_start(wq_sb[:], cp(wq))
                    wv_sb = pw.tile([P, NC, 512], F32R)
                    nc.sync.dma_start(wv_sb[:], cp(wv))
                nc.vector.tensor_copy(
                    V_sb[:, :, :, HD:HD + 1],
                    ones_c[:, None, :, None].to_broadcast([P, NTT, HL, 1]))
                for bb in range(NB):
                    xb = px.tile([P, NC, 512], F32R, tag="xb")
                    nc.sync.dma_start(xb[:], src_dram[bb][:])
                    for m in range(NCH):  # K and Q, mapping (b)
                        ps = pps.tile([P, 512], F32, tag="bank")
                        for c in range(NC):
                            nc.tensor.matmul(ps[:], lhsT=wk_sb[:, c, m * P:(m + 1) * P],
                                             rhs=xb[:, c, :], start=(c == 0),
                                             stop=(c == NC - 1))
                        tb = slice(bb * 512, (bb + 1) * 512)
                        if kbias is None:
                            nc.vector.tensor_copy(K_sb[:, m, tb], ps[:])
                        else:
                            nc.vector.tensor_scalar(K_sb[:, m, tb], ps[:],
                                                    kbias[:, m:m + 1], None, ADD)
                        ps = pps.tile([P, 512], F32, tag="bank")
                        for c in range(NC):
                            nc.tensor.matmul(ps[:], lhsT=wq_sb[:, c, m * P:(m + 1) * P],
                                             rhs=xb[:, c, :], start=(c == 0),
                                             stop=(c == NC - 1))
                        st = pst.tile([P, 512], F32R, tag="q")
                        if qbias is None:
                            nc.vector.tensor_copy(st[:], ps[:])
                        else:
                            nc.vector.tensor_scalar(st[:], ps[:],
                                                    qbias[:, m:m + 1], None, ADD)
                        nc.sync.dma_start(qd[bb][:, m, :], st[:])
                    for st_i in range(4):  # V, mapping (a): token-major
                        tt = bb * 4 + st_i
                        ps = pps.tile([P, 512], F32, tag="bank")
                        for c in range(NC):
                            nc.tensor.matmul(
                                ps[:], lhsT=xb[:, c, st_i * P:(st_i + 1) * P],
                                rhs=wv_sb[:, c, :], start=(c == 0), stop=(c == NC - 1))
                        if vbias is not None:
                            nc.vector.tensor_tensor(ps[:], ps[:], vbias[:], ADD)
                        nc.vector.tensor_copy(
                            V_sb[:, tt, :, 0:HD],
                            ps[:].rearrange("p (h d) -> p h d", h=HL))

        def attention(K_sb, V_sb, qd, wo, bo_half, apd, causal):
            """TP-heads attention for all T queries; writes partial
            out-projection (FM) to apd."""
            with tc.tile_pool(name="aw", bufs=1) as aw, \
                 tc.tile_pool(name="aq", bufs=1) as aq, \
                 tc.tile_pool(name="aa", bufs=2) as aa, \
                 tc.tile_pool(name="ao", bufs=1) as ao, \
                 tc.tile_pool(name="ast", bufs=3) as ast, \
                 tc.tile_pool(name="asc", bufs=2, space="PSUM") as asc, \
                 tc.tile_pool(name="abk", bufs=4, space="PSUM") as abk:
                wo_sb = aw.tile([P, NCH, E], F32R)
                nc.sync.dma_start(wo_sb[:], cp(wo))
                if causal:
                    mask_sb = aw.tile([P, 4, 512], F32)
                    nc.sync.dma_start(mask_sb[:], masks.rearrange("m p q -> p m q"))
                for qb in range(NB):
                    qblk = aq.tile([P, NCH, 512], F32R, tag="qblk")
                    nc.sync.dma_start(qblk[:], qd[qb][:])
                    o_blk = ao.tile([P, NCH, 512], F32, tag="oblk")
                    n_kt = (4 * qb + 4) if causal else NTT
                    for hp in range(NCH):
                        av = [abk.tile([65, 512], F32, tag="bank",
                                       name=f"av{qb}_{hp}_{j}") for j in (0, 1)]
                        for kt in range(n_kt):
                            sps = asc.tile([P, 2, 512], F32, tag="sc")
                            for j in (0, 1):
                                o = j * HD
                                nc.tensor.matmul(
                                    sps[:, j, :],
                                    lhsT=K_sb[o:o + HD, hp, kt * P:(kt + 1) * P],
                                    rhs=qblk[o:o + HD, hp, :],
                                    start=True, stop=True)
                            if causal and kt >= 4 * qb:
                                r = kt - 4 * qb
                                nc.vector.tensor_tensor(
                                    sps[:], sps[:],
                                    mask_sb[:, r:r + 1, :].to_broadcast([P, 2, 512]),
                                    ADD)
                            at = aa.tile([P, 2, 512], F32R, tag="at")
                            nc.scalar.activation(at[:], sps[:], Exp, scale=0.125)
                            for j in (0, 1):
                                nc.tensor.matmul(
                                    av[j][:], lhsT=V_sb[:, kt, 2 * hp + j, :],
                                    rhs=at[:, j, :], start=(kt == 0),
                                    stop=(kt == n_kt - 1))
                        for j in (0, 1):
                            rd = ast.tile([1, 512], F32R, tag="rd")
                            nc.vector.reciprocal(rd[:], av[j][64:65, :])
                            bc = abk.tile([64, 512], F32, tag="bank")
                            nc.tensor.matmul(bc[:], lhsT=ones_rr[0:1, 0:64],
                                             rhs=rd[:], start=True, stop=True)
                            otmp = ast.tile([64, 512], F32, tag="ot")
                            nc.vector.tensor_copy(otmp[:], av[j][0:64, :])
                            if j == 0:
                                nc.vector.tensor_tensor(o_blk[0:64, hp, :],
                                                        otmp[:], bc[:], MULT)
                            else:
                                o2 = ast.tile([64, 512], F32, tag="o2")
                                nc.vector.tensor_tensor(o2[:], otmp[:], bc[:], MULT)
                                nc.sync.dma_start(o_blk[64:128, hp, :], o2[:])
                    o_r = ao.tile([P, NCH, 512], F32R, tag="or")
                    nc.vector.tensor_scalar(o_r[:], o_blk[:], 0.0, None, ADD)
                    for m in range(NC):  # partial out-projection, mapping (b)
                        ps = abk.tile([P, 512], F32, tag="bank")
                        for c in range(NCH):
                            nc.tensor.matmul(ps[:], lhsT=wo_sb[:, c, m * P:(m + 1) * P],
                                             rhs=o_r[:, c, :], start=(c == 0),
                                             stop=(c == NCH - 1))
                        st = ast.tile([P, 512], F32, tag="st")
                        nc.vector.tensor_scalar(st[:], ps[:], bo_half[:, m:m + 1],
                                                None, ADD)
                        nc.sync.dma_start(apd[qb][:, m, :], st[:])

        def layernorm_block(xb, lnp, lnps, W=512, want_f32r=True, sbufs=1):
            """FM layernorm of one 512-token block. xb is F32.
            Returns (normalized F32 tile, F32R copy or None) — both pre-g/b."""
            xr = lnp.tile([P, NC, W], F32R, tag="xr", bufs=sbufs)
            nc.vector.tensor_scalar(xr[:], xb[:], 0.0, None, ADD)
            xsq = lnp.tile([P, NC, W], F32R, tag="xsq", bufs=sbufs)
            nc.vector.tensor_tensor(xsq[:], xb[:], xb[:], MULT)
            s_ps = lnps.tile([1, W], F32, tag="stat", bufs=2)
            q_ps = lnps.tile([1, W], F32, tag="stat", bufs=2)
            for c in range(NC):
                nc.tensor.matmul(s_ps[:], lhsT=ones_c[:], rhs=xr[:, c, :],
                                 start=(c == 0), stop=(c == NC - 1))
                nc.tensor.matmul(q_ps[:], lhsT=ones_c[:], rhs=xsq[:, c, :],
                                 start=(c == 0), stop=(c == NC - 1))
            m_t = lnp.tile([1, W], F32, tag="m")
            nc.vector.tensor_scalar(m_t[:], s_ps[:], 1.0 / E, None, MULT)
            mq_t = lnp.tile([1, W], F32, tag="mq")
            nc.vector.tensor_scalar(mq_t[:], q_ps[:], 1.0 / E, None, MULT)
            var_t = lnp.tile([1, W], F32, tag="var")
            nc.vector.tensor_tensor(var_t[:], m_t[:], m_t[:], MULT)
            nc.vector.tensor_tensor(var_t[:], mq_t[:], var_t[:], SUB)
            nc.vector.tensor_scalar(var_t[:], var_t[:], EPS, None, ADD)
            # rstd = rsqrt(var): quake seed + 2 Newton steps, all on DVE
            I32_ = mybir.dt.int32
            SHR = mybir.AluOpType.logical_shift_right
            rstd_t = lnp.tile([1, W], F32, tag="rstd")
            nc.vector.tensor_scalar(rstd_t[:].bitcast(I32_), var_t[:].bitcast(I32_),
                                    1, None, SHR)
            nc.vector.tensor_scalar(rstd_t[:].bitcast(I32_), rstd_t[:].bitcast(I32_),
                                    -1, 0x5f3759df, MULT, ADD)
            nt_t = lnp.tile([1, W], F32, tag="nt")
            for _ in range(2):
                nc.vector.tensor_tensor(nt_t[:], rstd_t[:], rstd_t[:], MULT)
                nc.vector.tensor_tensor(nt_t[:], nt_t[:], var_t[:], MULT)
                nc.vector.tensor_scalar(nt_t[:], nt_t[:], -0.5, 1.5, MULT, ADD)
                nc.vector.tensor_tensor(rstd_t[:], rstd_t[:], nt_t[:], MULT)
            mr_t = lnp.tile([1, W], F32, tag="mr")
            nc.vector.tensor_tensor(mr_t[:], m_t[:], rstd_t[:], MULT)
            rb = lnps.tile([P, W], F32, tag="bc", bufs=2)
            nc.tensor.matmul(rb[:], lhsT=ones_r[:], rhs=rstd_t[:],
                             start=True, stop=True)
            mb = lnps.tile([P, W], F32, tag="bc", bufs=2)
            nc.tensor.matmul(mb[:], lhsT=ones_r[:], rhs=mr_t[:],
                             start=True, stop=True)
            nb_t = lnp.tile([P, NC, W], F32, tag="nb", bufs=2)
            nc.vector.tensor_tensor(nb_t[:], xb[:],
                                    rb[:, None, :].to_broadcast([P, NC, W]), MULT)
            nc.vector.tensor_tensor(nb_t[:], nb_t[:],
                                    mb[:, None, :].to_broadcast([P, NC, W]), SUB)
            if not want_f32r:
                return nb_t, None
            nb_r = lnp.tile([P, NC, W], F32R, tag="nbr", bufs=2)
            nc.vector.tensor_scalar(nb_r[:], nb_t[:], 0.0, None, ADD)
            return nb_t, nb_r

        # ================= layer 1 ==========================================
        with tc.tile_pool(name="l1", bufs=1) as l1:
            K1 = l1.tile([P, NCH, T], F32R)
            V1 = l1.tile([P, NTT, HL, HD + 1], F32R)
            proj_pass(e_dram, wk1, wq1, wv1, K1, V1, q_dram[0],
                      None, None, None)
            attention(K1, V1, q_dram[0], wo1, bo1_sb, attn_p[0], causal=True)

        _allreduce(nc, tc, attn_p[0], attn_f[0], T, groups, fake_cc)

        # x1 = e + attn1_full (+bo1 already inside); LN1 -> n1_dram
        with tc.tile_pool(name="ln1", bufs=2) as lnp, \
             tc.tile_pool(name="ln1ps", bufs=2, space="PSUM") as lnps:
            W1 = 256
            for bb in range(T // W1):
                hs = slice((bb % 2) * W1, (bb % 2 + 1) * W1)
                eb = lnp.tile([P, NC, W1], F32R, tag="eb")
                nc.sync.dma_start(eb[:], e_dram[bb // 2][:, :, hs])
                ab = lnp.tile([P, NC, W1], F32, tag="ab")
                nc.sync.dma_start(ab[:], attn_f[0][bb // 2][:, :, hs])
                x1 = lnp.tile([P, NC, W1], F32, tag="x1")
                nc.vector.tensor_tensor(x1[:], _f32(eb[:]), ab[:], ADD)
                _, nb_r = layernorm_block(x1, lnp, lnps, W=W1)
                nc.sync.dma_start(n1_dram[bb // 2][:, :, hs], nb_r[:])

        # ================= layer 2 ==========================================
        with tc.tile_pool(name="l2", bufs=1) as l2:
            K2 = l2.tile([P, NCH, T], F32R)
            V2 = l2.tile([P, NTT, HL, HD + 1], F32R)
            proj_pass(n1_dram, wk2, wq2, wv2, K2, V2, q_dram[1],
                      kb2_sb, qb2_sb, vb2_b)
            attention(K2, V2, q_dram[1], wo2, bo2_sb, attn_p[1], causal=False)

        _allreduce(nc, tc, attn_p[1], attn_f[1], T, groups, fake_cc)

        # ========== x2 + LN2 + FFN + LN3 + output, fused per 256-tok block ====
        with tc.tile_pool(name="fw", bufs=1) as fwp, \
             tc.tile_pool(name="ff", bufs=1) as ffp, \
             tc.tile_pool(name="ffps", bufs=3, space="PSUM") as ffps:
            fw1_sb = fwp.tile([P, NC, 512], F32R)
            nc.sync.dma_start(fw1_sb[:], cp(fw1))
            fw2_sb = fwp.tile([P, NCH, E], F32R)
            nc.sync.dma_start(fw2_sb[:], cp(fw2))
            fw3_sb = fwp.tile([P, NC, 512], F32R)
            nc.sync.dma_start(fw3_sb[:], cp(fw3))
            fw4_sb = fwp.tile([P, NCH, E], F32R)
            nc.sync.dma_start(fw4_sb[:], cp(fw4))
            W = 256
            gbc = g_sb[:, :, None].to_broadcast([P, NC, W])
            lnbbc = lnb_sb[:, :, None].to_broadcast([P, NC, W])
            for bb in range(T // W):
                tb = slice(bb * W, (bb + 1) * W)
                n1b = ffp.tile([P, NC, W], F32R, tag="n1b", bufs=2)
                nc.sync.dma_start(
                    n1b[:], n1_dram[bb // 2][:, :, (bb % 2) * W:(bb % 2 + 1) * W])
                a2b = ffp.tile([P, NC, W], F32, tag="a2b", bufs=2)
                nc.sync.dma_start(
                    a2b[:], attn_f[1][bb // 2][:, :, (bb % 2) * W:(bb % 2 + 1) * W])
                x2 = ffp.tile([P, NC, W], F32, tag="xres", bufs=2)
                nc.vector.tensor_tensor(x2[:], _f32(n1b[:]), gbc, MULT)
                nc.vector.tensor_tensor(x2[:], x2[:], a2b[:], ADD)
                n2_f, n2 = layernorm_block(x2, ffp, ffps, W=W)
                h1 = ffp.tile([P, NCH, W], F32R, tag="hsm", bufs=2)
                for m in range(NCH):
                    ps = ffps.tile([P, W], F32, tag="bank")
                    for c in range(NC):
                        nc.tensor.matmul(ps[:], lhsT=fw1_sb[:, c, m * P:(m + 1) * P],
                                         rhs=n2[:, c, :], start=(c == 0),
                                         stop=(c == NC - 1))
                    nc.scalar.activation(h1[:, m, :], ps[:],
                                         mybir.ActivationFunctionType.Relu, bias=fb1_sb[:, m:m + 1])
                h2 = ffp.tile([P, NC, W], F32R, tag="h2", bufs=2)
                for m in range(NC):
                    ps = ffps.tile([P, W], F32, tag="bank")
                    for c in range(NCH):
                        nc.tensor.matmul(ps[:], lhsT=fw2_sb[:, c, m * P:(m + 1) * P],
                                         rhs=h1[:, c, :], start=(c == 0),
                                         stop=(c == NCH - 1))
                    nc.scalar.activation(h2[:, m, :], ps[:],
                                         mybir.ActivationFunctionType.Relu, bias=fb2_sb[:, m:m + 1])
                h3 = ffp.tile([P, NCH, W], F32R, tag="hsm", bufs=2)
                for m in range(NCH):
                    ps = ffps.tile([P, W], F32, tag="bank")
                    for c in range(NC):
                        nc.tensor.matmul(ps[:], lhsT=fw3_sb[:, c, m * P:(m + 1) * P],
                                         rhs=h2[:, c, :], start=(c == 0),
                                         stop=(c == NC - 1))
                    nc.scalar.activation(h3[:, m, :], ps[:],
                                         mybir.ActivationFunctionType.Relu, bias=fb3_sb[:, m:m + 1])
                x3 = ffp.tile([P, NC, W], F32, tag="xres", bufs=2)
                nc.vector.tensor_tensor(x3[:], n2_f[:], gbc, MULT)
                for m in range(NC):
                    ps = ffps.tile([P, W], F32, tag="bank")
                    for c in range(NCH):
                        nc.tensor.matmul(ps[:], lhsT=fw4_sb[:, c, m * P:(m + 1) * P],
                                         rhs=h3[:, c, :], start=(c == 0),
                                         stop=(c == NCH - 1))
                    nc.vector.tensor_scalar(ps[:], ps[:], fb4_sb[:, m:m + 1],
                                            None, ADD)
                    nc.vector.tensor_tensor(x3[:, m, :], x3[:, m, :], ps[:], ADD)
                n3_f, _ = layernorm_block(x3, ffp, ffps, W=W, want_f32r=False)
                r3 = ffp.tile([P, NC, W], F32, tag="a2b", bufs=2)
                nc.vector.tensor_tensor(r3[:], n3_f[:], gbc, MULT)
                nc.vector.tensor_tensor(r3[:], r3[:], lnbbc, ADD)
                for c in range(NC):
                    for st_i in range(W // P):
                        tp = ffps.tile([P, P], F32, tag="bank")
                        nc.tensor.transpose(tp[:], r3[:, c, st_i * P:(st_i + 1) * P],
                                            ident[:])
                        ost = ffp.tile([P, P], F32, tag="ost", bufs=4)
                        nc.scalar.copy(ost[:], tp[:])
                        nc.sync.dma_start(
                            out[bb * W + st_i * P: bb * W + (st_i + 1) * P,
                                c * P:(c + 1) * P], ost[:])

    nc.compile()
    return nc


def _allreduce(nc, tc, src_d, dst_d, T, groups, fake_cc):
    """Pairwise AllReduce, chunked along tokens so collectives overlap the
    producing compute. fake_cc models DMA volume for single-core TimelineSim."""
    NC_ = E // P
    if fake_cc:
        with tc.tile_pool(name="fcc", bufs=2) as fcc:
            for bb in range(T // 512):
                st = fcc.tile([P, NC_, 512], mybir.dt.float32, tag="st")
                nc.sync.dma_start(st[:], src_d[bb][:])
                nc.sync.dma_start(dst_d[bb][:], st[:])
        return
    for bb in range(T // 512):
        nc.gpsimd.collective_compute(
            "AllReduce", mybir.AluOpType.add, replica_groups=groups,
            ins=[src_d[bb][:].opt()], outs=[dst_d[bb][:].opt()])


_cache = {}


def _get_program(T, n_cores):
    key = (T, n_cores)
    if key not in _cache:
        _cache[key] = build_program(T, n_cores)
    return _cache[key]


def make_masks():
    m = np.zeros((4, P, 512), np.float32)
    for r in range(4):
        for k in range(P):
            m[r, k, :] = np.where(np.arange(512) >= (128 * r + k), 0.0, -1e9)
    return m


def build_in_maps(inputs, T, n_cores):
    f = lambda k: np.ascontiguousarray(np.asarray(inputs[k], dtype=np.float32))
    x = np.asarray(inputs["x"]).astype(np.int32)
    tok_emb, pos_emb = f("tok_emb"), f("pos_emb")
    ln_g, ln_b = f("ln_g"), f("ln_b")
    masks = make_masks()
    ones_col = np.ones((P, 1), np.float32)
    ones_row = np.ones((1, P), np.float32)
    in_maps = []
    for core in range(n_cores):
        b, hh = core // 2, core % 2
        hs = slice(hh * 512, (hh + 1) * 512)
        im = dict(
            ids=x[b, :T].reshape(T, 1),
            tok_emb=tok_emb,
            pos_emb=pos_emb[:T],
            wq1=np.ascontiguousarray(f("m_Wq")[:, hs]),
            wk1=np.ascontiguousarray(f("m_Wk")[:, hs]),
            wv1=np.ascontiguousarray(f("m_Wv")[:, hs]),
            wo1=np.ascontiguousarray(f("m_Wo")[hs, :]),
            wq2=np.ascontiguousarray((ln_g[:, None] * f("h_Wq"))[:, hs]),
            wk2=np.ascontiguousarray((ln_g[:, None] * f("h_Wk"))[:, hs]),
            wv2=np.ascontiguousarray((ln_g[:, None] * f("h_Wv"))[:, hs]),
            wo2=np.ascontiguousarray(f("h_Wo")[hs, :]),
            fw1=np.ascontiguousarray(ln_g[:, None] * f("f_W1")),
            fw2=f("f_W2"), fw3=f("f_W3"), fw4=f("f_W4"),
            bo1h=f("m_bo") / 2.0,
            bo2h=(f("h_bo") + ln_b) / 2.0,
            kb2=np.ascontiguousarray(ln_b @ f("h_Wk"))[hs],
            qb2=np.ascontiguousarray(ln_b @ f("h_Wq"))[hs],
            vb2=np.ascontiguousarray(ln_b @ f("h_Wv"))[hs],
            fb1c=(ln_b @ f("f_W1") + f("f_b1")),
            fb2c=f("f_b2"), fb3c=f("f_b3"),
            fb4c=(f("f_b4") + ln_b),
            gvec=ln_g, lnbv=ln_b,
            masks=masks, ones_col=ones_col, ones_row=ones_row,
        )
        in_maps.append({k: np.ascontiguousarray(v) for k, v in im.items()})
    return in_maps


def run(inputs, T=2048, n_cores=8):
    nc = _get_program(T, n_cores)
    in_maps = build_in_maps(inputs, T, n_cores)
    res = bass_utils.run_bass_kernel_spmd(nc, in_maps,
                                          core_ids=list(range(n_cores)))
    nb = n_cores // 2
    out = np.stack([res.results[2 * b]["out"] for b in range(nb)], axis=0)
    return out, res


def kernel(**inputs):
    out, _ = run(inputs, T=2048, n_cores=8)
    return out

